# revision 1
# baseline (speedup 1.0000x reference)
"""Trainium2 Bass kernel: GPT-2-style causal multi-head attention.

Problem: B=4, S=2048, D=1024, H=16 heads (head_dim 64), fp32.
  q/k/v = x @ W{q,k,v} + b{q,k,v}; causal softmax attention; out = attn_out @ Wo + bo.

Sharding (8 cores): tensor-parallel over heads - each core owns 2 heads
(128 feature dims). Wq/Wk/Wv column-sliced, Wo row-sliced per core. Each core
computes a partial o_proj output (transposed, [D, B*S]); the host sums the 8
partials, transposes, and adds bo.

Layout strategy on-chip: everything is kept transposed ([feature, seq]) so that
all matmul contractions have their contraction dim on SBUF partitions:
  x^T (via PE transpose) -> q^T/k^T/v^T = W^T x^T -> S^T = K^T^T... scores
  computed as S^T[k, q] tiles -> exp on ACT -> P^T -> out^T = V^T-ext @ P^T
  (with an appended ones column producing the softmax denominators) ->
  normalize -> o_proj out^T = Wo^T attnout^T.
"""

import sys
import os

sys.path.insert(0, "/opt/trn_rl_repo")

import numpy as np

import concourse.bass as bass
import concourse.bacc as bacc
import concourse.tile as tile
import concourse.mybir as mybir
from concourse.bass_utils import run_bass_kernel_spmd

F32 = mybir.dt.float32
F32R = mybir.dt.float32r

B, S, D, H = 4, 2048, 1024, 16
HD = D // H  # 64
N_CORES = 8
HPC = H // N_CORES  # heads per core = 2
J = HPC * HD  # per-core feature dims = 128
BS = B * S  # 8192
NB = S // 128  # 16 s-blocks per batch
NC = S // 512  # 4 chunks of 512 per batch

# fast (relaxed-precision) fp32 for the big matmuls; exact fp32 for transposes.
# fp32r operands must be produced pre-rounded, so every tile feeding an fp32r
# matmul is declared float32r and written by a rounding copy/activation.
MM_DT = F32R


def build_kernel():
    nc = bacc.Bacc(
        "TRN2", target_bir_lowering=False, debug=False, enable_asserts=False,
        num_devices=N_CORES,
    )

    x_d = nc.dram_tensor("x", [BS, D], F32, kind="ExternalInput").ap()
    wq_d = nc.dram_tensor("wq", [D, J], F32, kind="ExternalInput").ap()
    wk_d = nc.dram_tensor("wk", [D, J], F32, kind="ExternalInput").ap()
    wv_d = nc.dram_tensor("wv", [D, J], F32, kind="ExternalInput").ap()
    wo_d = nc.dram_tensor("wo", [J, D], F32, kind="ExternalInput").ap()
    bq_d = nc.dram_tensor("bq", [J], F32, kind="ExternalInput").ap()
    bk_d = nc.dram_tensor("bk", [J], F32, kind="ExternalInput").ap()
    bv_d = nc.dram_tensor("bv", [J], F32, kind="ExternalInput").ap()
    out_d = nc.dram_tensor("out_t", [D, BS], F32, kind="ExternalOutput").ap()

    with tile.TileContext(nc) as tc:
        _emit(tc, nc, x_d, wq_d, wk_d, wv_d, wo_d, bq_d, bk_d, bv_d, out_d)

    nc.compile()
    return nc


def _emit(tc, nc, x_d, wq_d, wk_d, wv_d, wo_d, bq_d, bk_d, bv_d, out_d):
    from contextlib import ExitStack

    ctx = ExitStack()
    with ctx:
        const = ctx.enter_context(tc.tile_pool(name="const", bufs=1))
        wpool = ctx.enter_context(tc.tile_pool(name="w", bufs=1))
        xpool = ctx.enter_context(tc.tile_pool(name="x", bufs=6))
        xtpool = ctx.enter_context(tc.tile_pool(name="xt", bufs=12))
        qkvpool = ctx.enter_context(tc.tile_pool(name="qkv", bufs=2))
        vepool = ctx.enter_context(tc.tile_pool(name="ve", bufs=4))
        ptpool = ctx.enter_context(tc.tile_pool(name="pt", bufs=3))
        aopool = ctx.enter_context(tc.tile_pool(name="ao", bufs=2))
        nrmpool = ctx.enter_context(tc.tile_pool(name="nrm", bufs=2))
        stgpool = ctx.enter_context(tc.tile_pool(name="stg", bufs=3))
        ps_st = ctx.enter_context(tc.tile_pool(name="ps_st", bufs=2, space="PSUM"))
        ps_acc = ctx.enter_context(tc.tile_pool(name="ps_acc", bufs=2, space="PSUM"))
        ps_mm = ctx.enter_context(tc.tile_pool(name="ps_mm", bufs=2, space="PSUM"))

        # --- constants ---------------------------------------------------
        # identity[p, f] = 1 if p == f else 0   (for PE transpose)
        ident = const.tile([128, 128], F32, tag="ident")
        nc.gpsimd.memset(ident[:], 1.0)
        nc.gpsimd.affine_select(
            ident[:], ident[:], pattern=[[1, 128]],
            compare_op=mybir.AluOpType.is_equal, fill=0.0,
            base=0, channel_multiplier=-1,
        )
        # fp32r copy of the identity for transposing fp32r tiles (v^T)
        ident_r = const.tile([128, 128], MM_DT, tag="ident_r")
        nc.vector.tensor_copy(ident_r[:], ident[:])
        # causal mask for diagonal 128x128 blocks of S^T[k, q]:
        # keep (1.0) where k <= q i.e. f - p >= 0
        mask_f = const.tile([128, 128], F32, tag="mask_f")
        nc.gpsimd.memset(mask_f[:], 1.0)
        nc.gpsimd.affine_select(
            mask_f[:], mask_f[:], pattern=[[1, 128]],
            compare_op=mybir.AluOpType.is_ge, fill=0.0,
            base=0, channel_multiplier=-1,
        )
        mask = const.tile([128, 128], MM_DT, tag="mask")
        nc.vector.tensor_copy(mask[:], mask_f[:])
        # fp32r ones column-vector group for the softmax-denominator columns
        ones_f = const.tile([128, 16], F32, tag="ones_f")
        nc.gpsimd.memset(ones_f[:], 1.0)
        ones16 = const.tile([128, 16], MM_DT, tag="ones16")
        nc.vector.tensor_copy(ones16[:], ones_f[:])
        # fp32r ones [128, 64] for the recip partition-broadcast matmul
        ones64f = const.tile([128, 64], F32, tag="ones64f")
        nc.gpsimd.memset(ones64f[:], 1.0)
        ones64 = const.tile([128, 64], MM_DT, tag="ones64")
        nc.vector.tensor_copy(ones64[:], ones64f[:])

        # --- weights -----------------------------------------------------
        # wq/wk/wv: [D, J] -> one [128, 1024] tile per projection (contraction
        # block ib at cols [128*ib, 128*ib+128)). DMA can't cast to fp32r, so
        # stage as fp32 then round with a DVE copy.
        w_tiles = {}
        for name, wd in (("q", wq_d), ("k", wk_d), ("v", wv_d)):
            stg = wpool.tile([128, D], F32, tag="wstg", name="wstg", bufs=2)
            for ib in range(8):
                nc.sync.dma_start(
                    stg[:, ib * 128:(ib + 1) * 128],
                    wd[ib * 128:(ib + 1) * 128, :])
            t = wpool.tile([128, D], MM_DT, tag=f"w{name}", name=f"w{name}")
            nc.vector.tensor_copy(t[:], stg[:])
            w_tiles[name] = t
        wo_stg = wpool.tile([J, D], F32, tag="wstg", name="wo_stg", bufs=2)
        nc.sync.dma_start(wo_stg[:], wo_d[:, :])
        wo_t = wpool.tile([J, D], MM_DT, tag="wo")
        nc.vector.tensor_copy(wo_t[:], wo_stg[:])

        bias = {}
        for name, bd in (("q", bq_d), ("k", bk_d), ("v", bv_d)):
            t = const.tile([J, 1], F32, tag=f"b{name}")
            nc.sync.dma_start(t[:], bd.rearrange("(p o) -> p o", o=1))
            bias[name] = t

        # --- per-batch pipeline -----------------------------------------
        for b in range(B):
            s0 = b * S  # row offset into x / out^T columns

            # projections: q^T/k^T/v^T [J=128, 2048] for this batch.
            # Per 512-wide chunk: load x, PE-transpose to x^T, then the three
            # projection matmuls consume (and release) the chunk's x^T tiles.
            proj = {
                name: qkvpool.tile([J, S], MM_DT, tag=f"{name}t", name=f"{name}t")
                for name in ("q", "k", "v")
            }
            for c in range(NC):
                x_t = []
                for si in range(4):
                    sb = 4 * c + si
                    t = xpool.tile([128, D], F32, tag="x", name="x")
                    nc.sync.dma_start(
                        t[:], x_d[s0 + sb * 128: s0 + (sb + 1) * 128, :])
                    x_t.append(t)
                xt = []
                for ib in range(8):
                    pst = ps_mm.tile([128, 512], F32, tag="ps_mm", name="pst")
                    for si in range(4):
                        nc.tensor.transpose(
                            pst[:, si * 128:(si + 1) * 128],
                            x_t[si][:, ib * 128:(ib + 1) * 128],
                            ident[:],
                        )
                    t = xtpool.tile([128, 512], MM_DT, tag="xt", name="xt")
                    nc.vector.tensor_copy(t[:], pst[:])
                    xt.append(t)
                for name in ("q", "k", "v"):
                    pacc = ps_mm.tile([128, 512], F32, tag="ps_mm", name="pacc")
                    for ib in range(8):
                        nc.tensor.matmul(
                            pacc[:],
                            w_tiles[name][:, ib * 128:(ib + 1) * 128],
                            xt[ib][:],
                            start=(ib == 0), stop=(ib == 7),
                        )
                    # copy PSUM -> SBUF with per-partition bias add (on ACT)
                    nc.scalar.activation(
                        proj[name][:, c * 512:(c + 1) * 512], pacc[:],
                        mybir.ActivationFunctionType.Identity,
                        bias=bias[name][:],
                    )
            qt, kt, vt = proj["q"], proj["k"], proj["v"]

            # V natural (per head, with ones column appended):
            # ve[h]: [128 k, 16*65], block kb at cols [65*kb, 65*kb+65),
            # col 65*kb+64 is the ones column (softmax denominator trick).
            ve = []
            for h in range(HPC):
                t = vepool.tile([128, NB * 65], MM_DT, tag="ve")
                # ones columns at 65*kb + 64 via one strided copy
                nc.vector.tensor_copy(
                    t[:].rearrange("p (nb c) -> p nb c", c=65)[:, :, 64:65],
                    ones16[:].rearrange("p (a o) -> p a o", o=1),
                )
                ve.append(t)
            for sb in range(NB):
                pst = ps_mm.tile([128, 512], F32, tag="ps_mm")
                nc.tensor.transpose(
                    pst[:, 0:128].bitcast(MM_DT),
                    vt[:, sb * 128:(sb + 1) * 128], ident_r[:],
                )
                for h in range(HPC):
                    nc.vector.tensor_copy(
                        ve[h][:, sb * 65: sb * 65 + 64],
                        pst[:, h * 64:(h + 1) * 64].bitcast(MM_DT),
                    )

            # attention for each head
            aot = aopool.tile([J, S], MM_DT, tag="aot")  # attnout^T, heads stacked
            for h in range(HPC):
                hp = slice(h * HD, (h + 1) * HD)  # partition range of this head
                for p in range(2):  # chunk-pair passes: chunks {2p, 2p+1}
                    acc = [
                        ps_acc.tile([128, 512], F32, tag="ps_acc", name="acc0"),
                        ps_acc.tile([128, 512], F32, tag="ps_acc", name="acc1"),
                    ]
                    n_kb = 8 * p + 8
                    for kb in range(n_kb):
                        lo = max(0, 128 * kb - 1024 * p)  # local col offset
                        st = ps_st.tile([128, 1024], F32, tag="ps_st")
                        for half in range(2):
                            hlo = max(lo, 512 * half)
                            hhi = 512 * (half + 1)
                            if hlo >= hhi:
                                continue
                            nc.tensor.matmul(
                                st[:, hlo:hhi],
                                kt[hp, kb * 128:(kb + 1) * 128],
                                qt[hp, 1024 * p + hlo: 1024 * p + hhi],
                                start=True, stop=True,
                            )
                        pt = ptpool.tile([128, 1024], MM_DT, tag="pt")
                        nc.scalar.activation(
                            pt[:, lo:1024], st[:, lo:1024],
                            mybir.ActivationFunctionType.Exp,
                            scale=0.125,
                        )
                        # diagonal block (only when it falls in this pass):
                        # mask the lower triangle
                        if 128 * kb - 1024 * p >= 0:
                            nc.vector.tensor_mul(
                                pt[:, lo:lo + 128], pt[:, lo:lo + 128], mask[:],
                            )
                        for half in range(2):
                            chunk = 2 * p + half
                            if kb > 4 * chunk + 3:
                                continue
                            hlo = max(lo, 512 * half)
                            hhi = 512 * (half + 1)
                            nc.tensor.matmul(
                                acc[half][0:65, hlo - 512 * half: 512],
                                ve[h][:, kb * 65: kb * 65 + 65],
                                pt[:, hlo:hhi],
                                start=(kb == 0), stop=(kb == 4 * chunk + 3),
                            )
                    # normalize: rows 0..63 = unnormalized out^T, row 64 = rowsum
                    for half in range(2):
                        chunk = 2 * p + half
                        rec = nrmpool.tile([128, 512], F32, tag="rec")
                        nc.vector.reciprocal(rec[64:65, :], acc[half][64:65, :])
                        rec_r = nrmpool.tile([128, 512], MM_DT, tag="rec_r")
                        nc.vector.tensor_copy(rec_r[64:65, :], rec[64:65, :])
                        # broadcast recip row to partitions 0..63 via ones-col
                        # matmul (gpsimd partition_broadcast is unreliable)
                        bcp = ps_mm.tile([64, 512], F32, tag="ps_mm", name="bcp")
                        nc.tensor.matmul(
                            bcp[:], ones64[64:65, :], rec_r[64:65, :],
                            start=True, stop=True,
                        )
                        bct = nrmpool.tile([128, 512], F32, tag="bct")
                        nc.vector.tensor_copy(bct[0:64, :], bcp[:])
                        if h == 0:
                            nc.vector.tensor_mul(
                                aot[0:64, chunk * 512:(chunk + 1) * 512],
                                acc[half][0:64, :], bct[0:64, :],
                            )
                        else:
                            tmp = nrmpool.tile([64, 512], MM_DT, tag="tmp")
                            nc.vector.tensor_mul(
                                tmp[:], acc[half][0:64, :], bct[0:64, :],
                            )
                            # partition shift 0-63 -> 64-127 via SBUF->SBUF DMA
                            nc.sync.dma_start(
                                aot[64:128, chunk * 512:(chunk + 1) * 512], tmp[:],
                            )

            # o_proj: out^T[o, s] partial = Wo_slice^T @ attnout^T
            for ob in range(8):
                stg = stgpool.tile([128, S], F32, tag="stg")
                for c in range(NC):
                    pst = ps_mm.tile([128, 512], F32, tag="ps_mm")
                    nc.tensor.matmul(
                        pst[:],
                        wo_t[:, ob * 128:(ob + 1) * 128],
                        aot[:, c * 512:(c + 1) * 512],
                        start=True, stop=True,
                    )
                    nc.vector.tensor_copy(stg[:, c * 512:(c + 1) * 512], pst[:])
                nc.sync.dma_start(
                    out_d[ob * 128:(ob + 1) * 128, s0: s0 + S], stg[:],
                )


_NC_CACHE = None


def _get_nc():
    global _NC_CACHE
    if _NC_CACHE is None:
        _NC_CACHE = build_kernel()
    return _NC_CACHE


def kernel(**inputs) -> np.ndarray:
    x = np.ascontiguousarray(
        np.asarray(inputs["hidden_states"], np.float32).reshape(BS, D))
    Wq = np.asarray(inputs["Wq"], np.float32)
    Wk = np.asarray(inputs["Wk"], np.float32)
    Wv = np.asarray(inputs["Wv"], np.float32)
    Wo = np.asarray(inputs["Wo"], np.float32)
    bq = np.asarray(inputs["bq"], np.float32)
    bk = np.asarray(inputs["bk"], np.float32)
    bv = np.asarray(inputs["bv"], np.float32)
    bo = np.asarray(inputs["bo"], np.float32)

    nc = _get_nc()
    in_maps = []
    for c in range(N_CORES):
        js = slice(c * J, (c + 1) * J)
        in_maps.append({
            "x": x,
            "wq": np.ascontiguousarray(Wq[:, js]),
            "wk": np.ascontiguousarray(Wk[:, js]),
            "wv": np.ascontiguousarray(Wv[:, js]),
            "wo": np.ascontiguousarray(Wo[js, :]),
            "bq": np.ascontiguousarray(bq[js]),
            "bk": np.ascontiguousarray(bk[js]),
            "bv": np.ascontiguousarray(bv[js]),
        })

    res = run_bass_kernel_spmd(nc, in_maps, core_ids=list(range(N_CORES)))
    out_t = np.zeros((D, BS), np.float64)
    for c in range(N_CORES):
        out_t += res.results[c]["out_t"].astype(np.float64)
    out = out_t.T.astype(np.float32) + bo[None, :]
    return out.reshape(B, S, D)


if __name__ == "__main__":
    rng = np.random.default_rng(0)
    ins = {
        "hidden_states": rng.standard_normal((B, S, D), np.float32),
        "Wq": rng.standard_normal((D, D), np.float32) * 0.02,
        "bq": np.zeros(D, np.float32),
        "Wk": rng.standard_normal((D, D), np.float32) * 0.02,
        "bk": np.zeros(D, np.float32),
        "Wv": rng.standard_normal((D, D), np.float32) * 0.02,
        "bv": np.zeros(D, np.float32),
        "Wo": rng.standard_normal((D, D), np.float32) * 0.02,
        "bo": np.zeros(D, np.float32),
    }
    out = kernel(**ins)
    print("out", out.shape, out.dtype, float(np.abs(out).mean()))



# revision 19
# speedup vs baseline: 1.4817x; 1.4817x over previous
"""Trainium2 Bass kernel: GPT-2-style causal multi-head attention.

Problem: B=4, S=2048, D=1024, H=16 heads (head_dim 64), fp32.
  q/k/v = x @ W{q,k,v} + b{q,k,v}; causal softmax attention; out = attn_out @ Wo + bo.

Sharding (8 cores): core c owns batch b = c//2 and head-group g = c%2
(8 heads = 512 feature dims). Wq/Wk/Wv column-sliced, Wo row-sliced per core.
Each core emits a partial o_proj output out_t [D, S] (transposed); the host
sums the pair of partials per batch, transposes, and adds the folded bias
bo' = bv @ Wo + bo (exact: softmax rows sum to 1, so attention(v + bv) =
attention(v) + bv, and the +bv term commutes through the o_proj matmul).

All matmuls run in bf16 (fp32 PSUM accumulation). x arrives pre-transposed
and pre-cast from the host as x^T [D, S] bf16, so no on-chip transposes:
  q^T/k^T = W^T-slices (stationary) x^T (moving)        [j, s] layout
  v       = x^T-slices (stationary) Wv (moving)         [s, j] natural layout
  scores  S^T[k, q] = k^T (stationary, K=64) q^T (moving) -- the two heads of
          a j-tile use partition ranges 0:64 / 64:128, so their score matmuls
          land on disjoint PE row-groups and execute concurrently.
  exp on ACT (PSUM->SBUF bf16), causal diagonal masked in-place on GpSimd.
  AV: attnout^T = v-slice+ones-col (stationary, M=65) P^T (moving); row 64
      accumulates the softmax denominator.
  normalize: reciprocal_approx_fast + K=1 ones-matmul partition-broadcast +
      DVE multiply -> aot bf16; o_proj = Wo-slices (stationary) aot (moving).
"""

import sys

sys.path.insert(0, "/opt/trn_rl_repo")

import numpy as np
import ml_dtypes

import concourse.bass as bass
import concourse.bacc as bacc
import concourse.tile as tile
import concourse.mybir as mybir
from concourse.bass_utils import run_bass_kernel_spmd

F32 = mybir.dt.float32
F32R = mybir.dt.float32r
BF16 = mybir.dt.bfloat16
BF16_NP = ml_dtypes.bfloat16

B, S, D, H = 4, 2048, 1024, 16
HD = D // H  # 64
N_CORES = 8
HPC = 8  # heads per core
J = HPC * HD  # per-core feature dims = 512
NJB = J // 128  # j-blocks per core = 4
NDB = D // 128  # d-blocks = 8
NSB = S // 128  # s-blocks = 16
NC = S // 512  # 512-chunks = 4
VW = 65  # v-tile stride per head: 64 cols + 1 ones col


def build_kernel(debug_dumps=False):
    nc = bacc.Bacc(
        "TRN2", target_bir_lowering=False, debug=False, enable_asserts=False,
        num_devices=N_CORES,
    )

    xt_d = nc.dram_tensor("xt", [D, S], BF16, kind="ExternalInput").ap()
    wq_d = nc.dram_tensor("wq", [D, J], BF16, kind="ExternalInput").ap()
    wk_d = nc.dram_tensor("wk", [D, J], BF16, kind="ExternalInput").ap()
    wv_d = nc.dram_tensor("wv", [D, J], BF16, kind="ExternalInput").ap()
    wo_d = nc.dram_tensor("wo", [J, D], BF16, kind="ExternalInput").ap()
    bq_d = nc.dram_tensor("bq", [128, NJB], F32, kind="ExternalInput").ap()
    bk_d = nc.dram_tensor("bk", [128, NJB], F32, kind="ExternalInput").ap()
    out_d = nc.dram_tensor("out_t", [D, S], F32, kind="ExternalOutput").ap()
    dumps = None
    if debug_dumps:
        dumps = {
            name: nc.dram_tensor(name, shape, BF16, kind="ExternalOutput").ap()
            for name, shape in (
                ("d_qt", [128, NJB * S]),
                ("d_kt", [128, NJB * S]),
                ("d_vt", [128, NSB * HPC * VW]),
                ("d_aot", [128, NJB * S]),
                ("d_ptA0", [128, 1024]),
                ("d_ptB0", [128, 1024]),
                ("d_ptA4", [128, 1024]),
            )
        }
        for name, shape in (
            ("d_accA0", [65, 512]),
            ("d_accA1", [65, 512]),
            ("d_accB0", [65, 512]),
            ("d_rec", [65, 512]),
            ("d_bct", [64, 512]),
        ):
            dumps[name] = nc.dram_tensor(
                name, shape, F32, kind="ExternalOutput").ap()

    with tile.TileContext(nc) as tc:
        _emit(tc, nc, xt_d, wq_d, wk_d, wv_d, wo_d, bq_d, bk_d, out_d, dumps)

    nc.compile()
    return nc


def _emit(tc, nc, xt_d, wq_d, wk_d, wv_d, wo_d, bq_d, bk_d, out_d, dumps=None):
    from contextlib import ExitStack

    EXP = mybir.ActivationFunctionType.Exp
    IDENT = mybir.ActivationFunctionType.Identity

    ctx = ExitStack()
    with ctx:
        const = ctx.enter_context(tc.tile_pool(name="const", bufs=1))
        wpool = ctx.enter_context(tc.tile_pool(name="w", bufs=1))
        xpool = ctx.enter_context(tc.tile_pool(name="x", bufs=1))
        qkpool = ctx.enter_context(tc.tile_pool(name="qk", bufs=1))
        vpool = ctx.enter_context(tc.tile_pool(name="v", bufs=1))
        aopool = ctx.enter_context(tc.tile_pool(name="ao", bufs=1))
        ptpool = ctx.enter_context(tc.tile_pool(name="pt", bufs=4))
        nrmpool = ctx.enter_context(tc.tile_pool(name="nrm", bufs=3))
        ogpool = ctx.enter_context(tc.tile_pool(name="og", bufs=2))
        # PSUM: ps slots 2x[128,1024] = 4 banks; acc 4x[65,512] = 4 banks.
        ps = ctx.enter_context(tc.tile_pool(name="ps", bufs=2, space="PSUM"))
        ps_acc = ctx.enter_context(tc.tile_pool(name="ps_acc", bufs=4, space="PSUM"))

        # --- constants ---------------------------------------------------
        # ones columns (bf16) for the v-tile softmax-denominator cols
        ones_v = const.tile([128, NSB * HPC], BF16, tag="ones_v")
        nc.gpsimd.memset(ones_v[:], 1.0)
        # bf16 ones for the recip partition-broadcast matmul; row 64 is used
        # so its base partition matches the denominator row of the acc tiles
        ones_bc = const.tile([65, 64], BF16, tag="ones_bc")
        nc.gpsimd.memset(ones_bc[:], 1.0)
        # causal mask for diagonal 128x128 blocks of S^T[k, q]: keep q >= k
        mask_f = const.tile([128, 128], F32, tag="mask_f")
        nc.gpsimd.memset(mask_f[:], 1.0)
        nc.gpsimd.affine_select(
            mask_f[:], mask_f[:], pattern=[[1, 128]],
            compare_op=mybir.AluOpType.is_ge, fill=0.0,
            base=0, channel_multiplier=-1,
        )
        mask = const.tile([128, 128], BF16, tag="mask")
        nc.vector.tensor_copy(mask[:], mask_f[:])

        # --- weights / biases -------------------------------------------
        w_t = {}
        for name, wd in (("q", wq_d), ("k", wk_d), ("v", wv_d)):
            t = wpool.tile([128, NDB * J], BF16, tag=f"w{name}")
            nc.sync.dma_start(
                t[:].rearrange("p (a j) -> p a j", j=J),
                wd.rearrange("(a p) j -> p a j", p=128),
            )
            w_t[name] = t
        wo_t = wpool.tile([128, NJB * D], BF16, tag="wo")
        nc.sync.dma_start(
            wo_t[:].rearrange("p (a o) -> p a o", o=D),
            wo_d.rearrange("(a p) o -> p a o", p=128),
        )
        bqt = const.tile([128, NJB], F32, tag="bq")
        nc.sync.dma_start(bqt[:], bq_d)
        bkt = const.tile([128, NJB], F32, tag="bk")
        nc.sync.dma_start(bkt[:], bk_d)

        # --- x^T ---------------------------------------------------------
        xt = xpool.tile([128, NDB * S], BF16, tag="xt")
        nc.sync.dma_start(
            xt[:].rearrange("p (a s) -> p a s", s=S),
            xt_d.rearrange("(a p) s -> p a s", p=128),
        )

        # --- v natural [s, j] with ones cols (16 s-block groups) ---------
        # v_t cols: s-block sb at [sb*8*VW, ...), head h at 65h..65h+64,
        # ones at 65h+64.
        v_t = vpool.tile([128, NSB * HPC * VW], BF16, tag="vt")
        nc.vector.tensor_copy(
            v_t[:].rearrange("p (sb h c) -> p (sb h) c", sb=NSB, c=VW)[:, :, 64:65],
            ones_v[:].rearrange("p (a o) -> p a o", o=1),
        )
        for sb in range(NSB):
            pv = ps.tile([128, 1024], F32, tag="ps", name="pv")
            for db in range(NDB):
                nc.tensor.matmul(
                    pv[:, 0:J],
                    xt[:, 2048 * db + 128 * sb: 2048 * db + 128 * (sb + 1)],
                    w_t["v"][:, J * db: J * (db + 1)],
                    start=(db == 0), stop=(db == NDB - 1),
                )
            nc.scalar.activation(
                v_t[:, sb * HPC * VW: (sb + 1) * HPC * VW].rearrange(
                    "p (h c) -> p h c", c=VW)[:, :, 0:64],
                pv[:, 0:J].rearrange("p (h c) -> p h c", c=64),
                IDENT,
            )

        # --- q^T / k^T [j, s] --------------------------------------------
        q_t = qkpool.tile([128, NJB * S], BF16, tag="qt")
        k_t = qkpool.tile([128, NJB * S], BF16, tag="kt")
        for jb in range(NJB):
            for name, dst, bias in (("q", q_t, bqt), ("k", k_t, bkt)):
                for c in range(NC):
                    pq = ps.tile([128, 1024], F32, tag="ps", name="pq")
                    for db in range(NDB):
                        nc.tensor.matmul(
                            pq[:, 0:512],
                            w_t[name][:, J * db + 128 * jb: J * db + 128 * (jb + 1)],
                            xt[:, 2048 * db + 512 * c: 2048 * db + 512 * (c + 1)],
                            start=(db == 0), stop=(db == NDB - 1),
                        )
                    nc.scalar.activation(
                        dst[:, S * jb + 512 * c: S * jb + 512 * (c + 1)],
                        pq[:, 0:512],
                        IDENT,
                        bias=bias[:, jb: jb + 1],
                    )

        # --- attention: 4 head-pairs (j-tiles), 2 q-halves each ----------
        aot = aopool.tile([128, NJB * S], BF16, tag="aot")
        for t in range(NJB):
            hA, hB = 2 * t, 2 * t + 1
            for p in range(2):
                acc = {}
                for X in ("A", "B"):
                    for half in range(2):
                        acc[(X, half)] = ps_acc.tile(
                            [65, 512], F32, tag="acc", name=f"acc{X}{half}")
                n_kb = 8 * p + 8
                for kb in range(n_kb):
                    lo = max(0, 128 * kb - 1024 * p)
                    rows = {"A": slice(0, 64), "B": slice(64, 128)}
                    sts = {
                        X: ps.tile([128, 1024], F32, tag="ps", name=f"st{X}")
                        for X in ("A", "B")
                    }
                    # interleave A/B halves so the two heads' K=64 matmuls
                    # (PE row-groups 0-1 vs 2-3) execute concurrently
                    for half in range(2):
                        hlo = max(lo, 512 * half)
                        if hlo >= 512 * (half + 1):
                            continue
                        for X in ("A", "B"):
                            nc.tensor.matmul(
                                sts[X][:, hlo: 512 * (half + 1)],
                                k_t[rows[X],
                                    S * t + 128 * kb: S * t + 128 * (kb + 1)],
                                q_t[rows[X],
                                    S * t + 1024 * p + hlo:
                                    S * t + 1024 * p + 512 * (half + 1)],
                                start=True, stop=True,
                            )
                    pts = {}
                    for X in ("A", "B"):
                        pt = ptpool.tile([128, 1024], BF16, tag="pt", name=f"pt{X}")
                        nc.scalar.activation(
                            pt[:, lo:1024], sts[X][:, lo:1024], EXP, scale=0.125,
                        )
                        # causal mask on the diagonal block: keep q >= k
                        if 128 * kb - 1024 * p >= 0:
                            nc.vector.tensor_mul(
                                pt[:, lo: lo + 128], pt[:, lo: lo + 128],
                                mask[:],
                            )
                        if (dumps is not None and t == 0 and p == 0
                                and (kb, X) in ((0, "A"), (0, "B"), (4, "A"))):
                            nc.sync.dma_start(
                                dumps[f"d_pt{X}{kb}"][:, :], pt[:])
                        pts[X] = pt
                    for X, h in (("A", hA), ("B", hB)):
                        for half in range(2):
                            chunk = 2 * p + half
                            if kb > 4 * chunk + 3:
                                continue
                            wlo = max(0, 128 * kb - 512 * chunk)
                            nc.tensor.matmul(
                                acc[(X, half)][0:65, wlo:512],
                                v_t[:, (8 * kb + h) * VW: (8 * kb + h) * VW + VW],
                                pts[X][:, 512 * half + wlo: 512 * (half + 1)],
                                start=(kb == 0), stop=(kb == 4 * chunk + 3),
                            )
                # normalize: rows 0..63 raw attnout^T, row 64 = denominator.
                # All DVE ops stay partition-aligned (rec row 64 -> row 64);
                # head B's rows land at aot partitions 64..127 via an
                # SBUF->SBUF DMA partition shift (DVE lanes can't cross).
                if dumps is not None and t == 0 and p == 0:
                    for nm, key in (("d_accA0", ("A", 0)), ("d_accA1", ("A", 1)),
                                    ("d_accB0", ("B", 0))):
                        dcp = nrmpool.tile([65, 512], F32, tag="dcp", bufs=3)
                        nc.vector.tensor_copy(dcp[:], acc[key][:])
                        nc.sync.dma_start(dumps[nm][:, :], dcp[:])
                for X, off in (("A", 0), ("B", 64)):
                    for half in range(2):
                        chunk = 2 * p + half
                        a = acc[(X, half)]
                        # Broadcast den to partitions 0..63 via a K=1 ones
                        # matmul, then take the reciprocal at base partition 0
                        # (reciprocal_approx_fast's custom-DVE uop mishandles
                        # PSUM sources and nonzero base partitions).
                        dnb = nrmpool.tile([65, 512], BF16, tag="dnb")
                        nc.vector.tensor_copy(dnb[64:65, :], a[64:65, :])
                        bcd = ps.tile([64, 512], F32, tag="ps", name="bcd")
                        nc.tensor.matmul(
                            bcd[:], ones_bc[64:65, :], dnb[64:65, :],
                            start=True, stop=True,
                        )
                        dnf = nrmpool.tile([64, 512], F32, tag="dnf")
                        nc.vector.tensor_copy(dnf[:], bcd[:])
                        bct = nrmpool.tile([64, 512], F32, tag="bct")
                        nc.vector.reciprocal_approx_fast(bct[:], dnf[:])
                        if (dumps is not None and t == 0 and p == 0
                                and X == "A" and half == 0):
                            nc.sync.dma_start(dumps["d_rec"][64:65, :],
                                              dnf[0:1, :])
                            nc.sync.dma_start(dumps["d_bct"][:, :], bct[:])
                        if off == 0:
                            nc.vector.tensor_mul(
                                aot[0:64,
                                    S * t + 512 * chunk: S * t + 512 * (chunk + 1)],
                                a[0:64, :], bct[:],
                            )
                        else:
                            tmp = nrmpool.tile([64, 512], BF16, tag="tmp")
                            nc.vector.tensor_mul(tmp[:], a[0:64, :], bct[:])
                            nc.sync.dma_start(
                                aot[64:128,
                                    S * t + 512 * chunk: S * t + 512 * (chunk + 1)],
                                tmp[:],
                            )

        if dumps is not None:
            nc.sync.dma_start(dumps["d_qt"][:, :], q_t[:])
            nc.sync.dma_start(dumps["d_kt"][:, :], k_t[:])
            nc.sync.dma_start(dumps["d_vt"][:, :], v_t[:])
            nc.sync.dma_start(dumps["d_aot"][:, :], aot[:])

        # --- o_proj: out^T[o, s] partial = Wo_slice^T @ aot ---------------
        for ob in range(NDB):
            og = ogpool.tile([128, S], F32, tag="og")
            for c in range(NC):
                po = ps.tile([128, 1024], F32, tag="ps", name="po")
                for jb in range(NJB):
                    nc.tensor.matmul(
                        po[:, 0:512],
                        wo_t[:, D * jb + 128 * ob: D * jb + 128 * (ob + 1)],
                        aot[:, S * jb + 512 * c: S * jb + 512 * (c + 1)],
                        start=(jb == 0), stop=(jb == NJB - 1),
                    )
                if c % 2 == 0:
                    nc.scalar.copy(og[:, 512 * c: 512 * (c + 1)], po[:, 0:512])
                else:
                    nc.vector.tensor_copy(og[:, 512 * c: 512 * (c + 1)], po[:, 0:512])
            nc.sync.dma_start(out_d[128 * ob: 128 * (ob + 1), :], og[:])


_NC_CACHE = None


def _get_nc():
    global _NC_CACHE
    if _NC_CACHE is None:
        _NC_CACHE = build_kernel()
    return _NC_CACHE


def build_in_maps(inputs):
    """Host-side sharding: per-core input dict for run_bass_kernel_spmd."""
    x = np.asarray(inputs["hidden_states"], np.float32)
    xt_b = [
        np.ascontiguousarray(x[b].T.astype(BF16_NP)) for b in range(B)
    ]  # [D, S] bf16 per batch
    Wq = np.asarray(inputs["Wq"], np.float32)
    Wk = np.asarray(inputs["Wk"], np.float32)
    Wv = np.asarray(inputs["Wv"], np.float32)
    Wo = np.asarray(inputs["Wo"], np.float32)
    bq = np.asarray(inputs["bq"], np.float32)
    bk = np.asarray(inputs["bk"], np.float32)

    in_maps = []
    for c in range(N_CORES):
        b, g = c // 2, c % 2
        js = slice(g * J, (g + 1) * J)
        in_maps.append({
            "xt": xt_b[b],
            "wq": np.ascontiguousarray(Wq[:, js].astype(BF16_NP)),
            "wk": np.ascontiguousarray(Wk[:, js].astype(BF16_NP)),
            "wv": np.ascontiguousarray(Wv[:, js].astype(BF16_NP)),
            "wo": np.ascontiguousarray(Wo[js, :].astype(BF16_NP)),
            "bq": np.ascontiguousarray(bq[js].reshape(NJB, 128).T),
            "bk": np.ascontiguousarray(bk[js].reshape(NJB, 128).T),
        })
    return in_maps


def assemble_output(results, inputs):
    """Sum per-batch partial pairs, transpose, add folded bias."""
    Wo = np.asarray(inputs["Wo"], np.float32)
    bv = np.asarray(inputs["bv"], np.float32)
    bo = np.asarray(inputs["bo"], np.float32)
    bo_f = bv @ Wo + bo
    out = np.empty((B, S, D), np.float32)
    for b in range(B):
        acc = results[2 * b]["out_t"].astype(np.float32) + \
            results[2 * b + 1]["out_t"].astype(np.float32)
        out[b] = acc.T + bo_f[None, :]
    return out


def kernel(**inputs) -> np.ndarray:
    nc = _get_nc()
    in_maps = build_in_maps(inputs)
    res = run_bass_kernel_spmd(nc, in_maps, core_ids=list(range(N_CORES)))
    return assemble_output(res.results, inputs)


if __name__ == "__main__":
    rng = np.random.default_rng(0)
    ins = {
        "hidden_states": rng.standard_normal((B, S, D)).astype(np.float32),
        "Wq": (rng.standard_normal((D, D)) * 0.02).astype(np.float32),
        "bq": np.zeros(D, np.float32),
        "Wk": (rng.standard_normal((D, D)) * 0.02).astype(np.float32),
        "bk": np.zeros(D, np.float32),
        "Wv": (rng.standard_normal((D, D)) * 0.02).astype(np.float32),
        "bv": np.zeros(D, np.float32),
        "Wo": (rng.standard_normal((D, D)) * 0.02).astype(np.float32),
        "bo": np.zeros(D, np.float32),
    }
    out = kernel(**ins)
    print("out", out.shape, out.dtype, float(np.abs(out).mean()))


# revision 24
# speedup vs baseline: 2.0313x; 1.3709x over previous
"""Trainium2 Bass kernel: GPT-2-style causal multi-head attention.

Problem: B=4, S=2048, D=1024, H=16 heads (head_dim 64), fp32.
  q/k/v = x @ W{q,k,v} + b{q,k,v}; causal softmax attention; out = attn_out @ Wo + bo.

Sharding (8 cores): core c owns batch b = c//2 and head-group g = c%2
(8 heads = 512 feature dims). Wq/Wk/Wv column-sliced, Wo row-sliced per core.
Each core emits a partial o_proj output out_t [D, S] (transposed); the host
sums the pair of partials per batch, transposes, and adds the folded bias
bo' = bv @ Wo + bo (exact: softmax rows sum to 1, so attention(v + bv) =
attention(v) + bv, and the +bv term commutes through the o_proj matmul).

All matmuls run in bf16 (fp32 PSUM accumulation). x arrives pre-transposed
and pre-cast from the host as x^T [D, S] bf16, so no on-chip transposes:
  q^T/k^T = W^T-slices (stationary) x^T (moving)        [j, s] layout
  v       = x^T-slices (stationary) Wv (moving)         [s, j] natural layout
  scores  S^T[k, q] = k^T (stationary, K=64) q^T (moving) -- the two heads of
          a j-tile use partition ranges 0:64 / 64:128, so their score matmuls
          land on disjoint PE row-groups and execute concurrently.
  exp on ACT (PSUM->SBUF bf16), causal diagonal masked in-place on GpSimd.
  AV: attnout^T = v-slice+ones-col (stationary, M=65) P^T (moving); row 64
      accumulates the softmax denominator.
  normalize: reciprocal_approx_fast + K=1 ones-matmul partition-broadcast +
      DVE multiply -> aot bf16; o_proj = Wo-slices (stationary) aot (moving).
"""

import sys

sys.path.insert(0, "/opt/trn_rl_repo")

import numpy as np
import ml_dtypes

import concourse.bass as bass
import concourse.bacc as bacc
import concourse.tile as tile
import concourse.mybir as mybir
from concourse.bass_utils import run_bass_kernel_spmd

F32 = mybir.dt.float32
F32R = mybir.dt.float32r
BF16 = mybir.dt.bfloat16
BF16_NP = ml_dtypes.bfloat16

B, S, D, H = 4, 2048, 1024, 16
HD = D // H  # 64
N_CORES = 8
HPC = 8  # heads per core
J = HPC * HD  # per-core feature dims = 512
NJB = J // 128  # j-blocks per core = 4
NDB = D // 128  # d-blocks = 8
NSB = S // 128  # s-blocks = 16
NC = S // 512  # 512-chunks = 4
VW = 65  # v-tile stride per head: 64 cols + 1 ones col


def build_kernel(debug_dumps=False):
    nc = bacc.Bacc(
        "TRN2", target_bir_lowering=False, debug=False, enable_asserts=False,
        num_devices=N_CORES,
    )

    xt_d = nc.dram_tensor("xt", [D, S], BF16, kind="ExternalInput").ap()
    wq_d = nc.dram_tensor("wq", [D, J], BF16, kind="ExternalInput").ap()
    wk_d = nc.dram_tensor("wk", [D, J], BF16, kind="ExternalInput").ap()
    wv_d = nc.dram_tensor("wv", [D, J], BF16, kind="ExternalInput").ap()
    wo_d = nc.dram_tensor("wo", [J, D], BF16, kind="ExternalInput").ap()
    bq_d = nc.dram_tensor("bq", [128, NJB], F32, kind="ExternalInput").ap()
    bk_d = nc.dram_tensor("bk", [128, NJB], F32, kind="ExternalInput").ap()
    out_d = nc.dram_tensor("out_t", [D, S], F32, kind="ExternalOutput").ap()
    dumps = None
    if debug_dumps:
        dumps = {
            name: nc.dram_tensor(name, shape, BF16, kind="ExternalOutput").ap()
            for name, shape in (
                ("d_qt", [128, NJB * S]),
                ("d_kt", [128, NJB * S]),
                ("d_vt", [128, NSB * HPC * VW]),
                ("d_aot", [128, NJB * S]),
            )
        }

    with tile.TileContext(nc) as tc:
        _emit(tc, nc, xt_d, wq_d, wk_d, wv_d, wo_d, bq_d, bk_d, out_d, dumps)

    nc.compile()
    return nc


def _emit(tc, nc, xt_d, wq_d, wk_d, wv_d, wo_d, bq_d, bk_d, out_d, dumps=None):
    from contextlib import ExitStack

    EXP = mybir.ActivationFunctionType.Exp
    IDENT = mybir.ActivationFunctionType.Identity

    ctx = ExitStack()
    with ctx:
        const = ctx.enter_context(tc.tile_pool(name="const", bufs=1))
        wpool = ctx.enter_context(tc.tile_pool(name="w", bufs=1))
        xpool = ctx.enter_context(tc.tile_pool(name="x", bufs=1))
        qkpool = ctx.enter_context(tc.tile_pool(name="qk", bufs=1))
        vpool = ctx.enter_context(tc.tile_pool(name="v", bufs=1))
        aopool = ctx.enter_context(tc.tile_pool(name="ao", bufs=1))
        ptpool = ctx.enter_context(tc.tile_pool(name="pt", bufs=4))
        nrmpool = ctx.enter_context(tc.tile_pool(name="nrm", bufs=3))
        ogpool = ctx.enter_context(tc.tile_pool(name="og", bufs=2))
        # PSUM: ps slots 3x[128,1024] = 6 banks; acc 2x[65,512] = 2 banks.
        ps = ctx.enter_context(tc.tile_pool(name="ps", bufs=3, space="PSUM"))
        ps_acc = ctx.enter_context(tc.tile_pool(name="ps_acc", bufs=2, space="PSUM"))

        # --- constants ---------------------------------------------------
        # ones columns (bf16) for the v-tile softmax-denominator cols
        ones_v = const.tile([128, NSB * HPC], BF16, tag="ones_v")
        nc.gpsimd.memset(ones_v[:], 1.0)
        # bf16 ones for the recip partition-broadcast matmul; row 64 is used
        # so its base partition matches the denominator row of the acc tiles
        ones_bc = const.tile([65, 64], BF16, tag="ones_bc")
        nc.gpsimd.memset(ones_bc[:], 1.0)
        # causal mask for diagonal 128x128 blocks of S^T[k, q]: keep q >= k
        mask_f = const.tile([128, 128], F32, tag="mask_f")
        nc.gpsimd.memset(mask_f[:], 1.0)
        nc.gpsimd.affine_select(
            mask_f[:], mask_f[:], pattern=[[1, 128]],
            compare_op=mybir.AluOpType.is_ge, fill=0.0,
            base=0, channel_multiplier=-1,
        )
        mask = const.tile([128, 128], BF16, tag="mask")
        nc.vector.tensor_copy(mask[:], mask_f[:])

        # --- weights / biases -------------------------------------------
        w_t = {}
        for name, wd in (("q", wq_d), ("k", wk_d), ("v", wv_d)):
            t = wpool.tile([128, NDB * J], BF16, tag=f"w{name}")
            nc.sync.dma_start(
                t[:].rearrange("p (a j) -> p a j", j=J),
                wd.rearrange("(a p) j -> p a j", p=128),
            )
            w_t[name] = t
        wo_t = wpool.tile([128, NJB * D], BF16, tag="wo")
        nc.sync.dma_start(
            wo_t[:].rearrange("p (a o) -> p a o", o=D),
            wo_d.rearrange("(a p) o -> p a o", p=128),
        )
        bqt = const.tile([128, NJB], F32, tag="bq")
        nc.sync.dma_start(bqt[:], bq_d)
        bkt = const.tile([128, NJB], F32, tag="bk")
        nc.sync.dma_start(bkt[:], bk_d)

        # --- x^T ---------------------------------------------------------
        xt = xpool.tile([128, NDB * S], BF16, tag="xt")
        nc.sync.dma_start(
            xt[:].rearrange("p (a s) -> p a s", s=S),
            xt_d.rearrange("(a p) s -> p a s", p=128),
        )

        # --- v natural [s, j] with ones cols (16 s-block groups) ---------
        # v_t cols: s-block sb at [sb*8*VW, ...), head h at 65h..65h+64,
        # ones at 65h+64.
        v_t = vpool.tile([128, NSB * HPC * VW], BF16, tag="vt")
        nc.vector.tensor_copy(
            v_t[:].rearrange("p (sb h c) -> p (sb h) c", sb=NSB, c=VW)[:, :, 64:65],
            ones_v[:].rearrange("p (a o) -> p a o", o=1),
        )
        for sb in range(NSB):
            pv = ps.tile([128, 1024], F32, tag="ps", name="pv")
            for db in range(NDB):
                nc.tensor.matmul(
                    pv[:, 0:J],
                    xt[:, 2048 * db + 128 * sb: 2048 * db + 128 * (sb + 1)],
                    w_t["v"][:, J * db: J * (db + 1)],
                    start=(db == 0), stop=(db == NDB - 1),
                )
            nc.scalar.activation(
                v_t[:, sb * HPC * VW: (sb + 1) * HPC * VW].rearrange(
                    "p (h c) -> p h c", c=VW)[:, :, 0:64],
                pv[:, 0:J].rearrange("p (h c) -> p h c", c=64),
                IDENT,
            )

        # --- q^T / k^T [j, s] --------------------------------------------
        q_t = qkpool.tile([128, NJB * S], BF16, tag="qt")
        k_t = qkpool.tile([128, NJB * S], BF16, tag="kt")
        for jb in range(NJB):
            for name, dst, bias in (("q", q_t, bqt), ("k", k_t, bkt)):
                for c in range(NC):
                    pq = ps.tile([128, 1024], F32, tag="ps", name="pq")
                    for db in range(NDB):
                        nc.tensor.matmul(
                            pq[:, 0:512],
                            w_t[name][:, J * db + 128 * jb: J * db + 128 * (jb + 1)],
                            xt[:, 2048 * db + 512 * c: 2048 * db + 512 * (c + 1)],
                            start=(db == 0), stop=(db == NDB - 1),
                        )
                    nc.scalar.activation(
                        dst[:, S * jb + 512 * c: S * jb + 512 * (c + 1)],
                        pq[:, 0:512],
                        IDENT,
                        bias=bias[:, jb: jb + 1],
                    )

        # --- attention: 4 head-pairs (j-tiles), 4 q-chunks each ----------
        # Per (pair, chunk, kb): one combined score tile holds head A in cols
        # 0:512 and head B in 512:1024 (separate PSUM banks -> the two K=64
        # matmuls run on disjoint PE row-groups concurrently); one 2D-AP exp
        # covers both heads; AV accumulates per head into [65, 512] accs.
        aot = aopool.tile([128, NJB * S], BF16, tag="aot")
        rows = {"A": slice(0, 64), "B": slice(64, 128)}
        for t in range(NJB):
            head = {"A": 2 * t, "B": 2 * t + 1}
            for c in range(NC):
                acc = {
                    X: ps_acc.tile([65, 512], F32, tag="acc", name=f"acc{X}")
                    for X in ("A", "B")
                }
                n_kb = 4 * c + 4
                for kb in range(n_kb):
                    wlo = max(0, 128 * kb - 512 * c)
                    st = ps.tile([128, 1024], F32, tag="ps", name="st")
                    for X in ("A", "B"):
                        nc.tensor.matmul(
                            st[:, 512 * (X == "B") + wlo:
                               512 * (X == "B") + 512],
                            k_t[rows[X],
                                S * t + 128 * kb: S * t + 128 * (kb + 1)],
                            q_t[rows[X],
                                S * t + 512 * c + wlo: S * t + 512 * (c + 1)],
                            start=True, stop=True,
                        )
                    pt = ptpool.tile([128, 1024], BF16, tag="pt", name="pt")
                    nc.scalar.activation(
                        pt[:].rearrange("p (x q) -> p x q", x=2)[:, :, wlo:512],
                        st[:].rearrange("p (x q) -> p x q", x=2)[:, :, wlo:512],
                        EXP, scale=0.125,
                    )
                    # causal mask on the diagonal 128x128 block: keep q >= k
                    if kb >= 4 * c:
                        for X in ("A", "B"):
                            o = 512 * (X == "B") + wlo
                            nc.vector.tensor_mul(
                                pt[:, o: o + 128], pt[:, o: o + 128], mask[:],
                            )
                    for X in ("A", "B"):
                        nc.tensor.matmul(
                            acc[X][0:65, wlo:512],
                            v_t[:, (8 * kb + head[X]) * VW:
                                (8 * kb + head[X]) * VW + VW],
                            pt[:, 512 * (X == "B") + wlo:
                               512 * (X == "B") + 512],
                            start=(kb == 0), stop=(kb == n_kb - 1),
                        )
                # normalize: rows 0..63 raw attnout^T, row 64 = denominator.
                # Broadcast den to partitions 0..63 via a K=1 ones matmul,
                # then reciprocal at base partition 0 (reciprocal_approx_fast
                # mishandles PSUM sources and nonzero base partitions).
                # Head B lands at aot partitions 64..127 via an SBUF->SBUF
                # DMA partition shift (DVE lanes can't cross partitions).
                for X, off in (("A", 0), ("B", 64)):
                    a = acc[X]
                    dnb = nrmpool.tile([65, 512], BF16, tag="dnb")
                    nc.vector.tensor_copy(dnb[64:65, :], a[64:65, :])
                    bcd = ps.tile([64, 512], F32, tag="ps", name="bcd")
                    nc.tensor.matmul(
                        bcd[:], ones_bc[64:65, :], dnb[64:65, :],
                        start=True, stop=True,
                    )
                    dnf = nrmpool.tile([64, 512], F32, tag="dnf")
                    nc.vector.tensor_copy(dnf[:], bcd[:])
                    bct = nrmpool.tile([64, 512], F32, tag="bct")
                    nc.vector.reciprocal_approx_fast(bct[:], dnf[:])
                    if off == 0:
                        nc.vector.tensor_mul(
                            aot[0:64,
                                S * t + 512 * c: S * t + 512 * (c + 1)],
                            a[0:64, :], bct[:],
                        )
                    else:
                        tmp = nrmpool.tile([64, 512], BF16, tag="tmp")
                        nc.vector.tensor_mul(tmp[:], a[0:64, :], bct[:])
                        nc.sync.dma_start(
                            aot[64:128,
                                S * t + 512 * c: S * t + 512 * (c + 1)],
                            tmp[:],
                        )

        if dumps is not None:
            nc.sync.dma_start(dumps["d_qt"][:, :], q_t[:])
            nc.sync.dma_start(dumps["d_kt"][:, :], k_t[:])
            nc.sync.dma_start(dumps["d_vt"][:, :], v_t[:])
            nc.sync.dma_start(dumps["d_aot"][:, :], aot[:])

        # --- o_proj: out^T[o, s] partial = Wo_slice^T @ aot ---------------
        for ob in range(NDB):
            og = ogpool.tile([128, S], F32, tag="og")
            for c in range(NC):
                po = ps.tile([128, 1024], F32, tag="ps", name="po")
                for jb in range(NJB):
                    nc.tensor.matmul(
                        po[:, 0:512],
                        wo_t[:, D * jb + 128 * ob: D * jb + 128 * (ob + 1)],
                        aot[:, S * jb + 512 * c: S * jb + 512 * (c + 1)],
                        start=(jb == 0), stop=(jb == NJB - 1),
                    )
                if c % 2 == 0:
                    nc.scalar.copy(og[:, 512 * c: 512 * (c + 1)], po[:, 0:512])
                else:
                    nc.vector.tensor_copy(og[:, 512 * c: 512 * (c + 1)], po[:, 0:512])
            nc.sync.dma_start(out_d[128 * ob: 128 * (ob + 1), :], og[:])


_NC_CACHE = None


def _get_nc():
    global _NC_CACHE
    if _NC_CACHE is None:
        _NC_CACHE = build_kernel()
    return _NC_CACHE


def build_in_maps(inputs):
    """Host-side sharding: per-core input dict for run_bass_kernel_spmd."""
    x = np.asarray(inputs["hidden_states"], np.float32)
    xt_b = [
        np.ascontiguousarray(x[b].T.astype(BF16_NP)) for b in range(B)
    ]  # [D, S] bf16 per batch
    Wq = np.asarray(inputs["Wq"], np.float32)
    Wk = np.asarray(inputs["Wk"], np.float32)
    Wv = np.asarray(inputs["Wv"], np.float32)
    Wo = np.asarray(inputs["Wo"], np.float32)
    bq = np.asarray(inputs["bq"], np.float32)
    bk = np.asarray(inputs["bk"], np.float32)

    in_maps = []
    for c in range(N_CORES):
        b, g = c // 2, c % 2
        js = slice(g * J, (g + 1) * J)
        in_maps.append({
            "xt": xt_b[b],
            "wq": np.ascontiguousarray(Wq[:, js].astype(BF16_NP)),
            "wk": np.ascontiguousarray(Wk[:, js].astype(BF16_NP)),
            "wv": np.ascontiguousarray(Wv[:, js].astype(BF16_NP)),
            "wo": np.ascontiguousarray(Wo[js, :].astype(BF16_NP)),
            "bq": np.ascontiguousarray(bq[js].reshape(NJB, 128).T),
            "bk": np.ascontiguousarray(bk[js].reshape(NJB, 128).T),
        })
    return in_maps


def assemble_output(results, inputs):
    """Sum per-batch partial pairs, transpose, add folded bias."""
    Wo = np.asarray(inputs["Wo"], np.float32)
    bv = np.asarray(inputs["bv"], np.float32)
    bo = np.asarray(inputs["bo"], np.float32)
    bo_f = bv @ Wo + bo
    out = np.empty((B, S, D), np.float32)
    for b in range(B):
        acc = results[2 * b]["out_t"].astype(np.float32) + \
            results[2 * b + 1]["out_t"].astype(np.float32)
        out[b] = acc.T + bo_f[None, :]
    return out


def kernel(**inputs) -> np.ndarray:
    nc = _get_nc()
    in_maps = build_in_maps(inputs)
    res = run_bass_kernel_spmd(nc, in_maps, core_ids=list(range(N_CORES)))
    return assemble_output(res.results, inputs)


if __name__ == "__main__":
    rng = np.random.default_rng(0)
    ins = {
        "hidden_states": rng.standard_normal((B, S, D)).astype(np.float32),
        "Wq": (rng.standard_normal((D, D)) * 0.02).astype(np.float32),
        "bq": np.zeros(D, np.float32),
        "Wk": (rng.standard_normal((D, D)) * 0.02).astype(np.float32),
        "bk": np.zeros(D, np.float32),
        "Wv": (rng.standard_normal((D, D)) * 0.02).astype(np.float32),
        "bv": np.zeros(D, np.float32),
        "Wo": (rng.standard_normal((D, D)) * 0.02).astype(np.float32),
        "bo": np.zeros(D, np.float32),
    }
    out = kernel(**ins)
    print("out", out.shape, out.dtype, float(np.abs(out).mean()))


# revision 25
# speedup vs baseline: 2.0543x; 1.0113x over previous
"""Trainium2 Bass kernel: GPT-2-style causal multi-head attention.

Problem: B=4, S=2048, D=1024, H=16 heads (head_dim 64), fp32.
  q/k/v = x @ W{q,k,v} + b{q,k,v}; causal softmax attention; out = attn_out @ Wo + bo.

Sharding (8 cores): core c owns batch b = c//2 and head-group g = c%2
(8 heads = 512 feature dims). Wq/Wk/Wv column-sliced, Wo row-sliced per core.
Each core emits a partial o_proj output out_t [D, S] (transposed); the host
sums the pair of partials per batch, transposes, and adds the folded bias
bo' = bv @ Wo + bo (exact: softmax rows sum to 1, so attention(v + bv) =
attention(v) + bv, and the +bv term commutes through the o_proj matmul).

All matmuls run in bf16 (fp32 PSUM accumulation). x arrives pre-transposed
and pre-cast from the host as x^T [D, S] bf16, so no on-chip transposes:
  q^T/k^T = W^T-slices (stationary) x^T (moving)        [j, s] layout
  v       = x^T-slices (stationary) Wv (moving)         [s, j] natural layout
  scores  S^T[k, q] = k^T (stationary, K=64) q^T (moving) -- the two heads of
          a j-tile use partition ranges 0:64 / 64:128, so their score matmuls
          land on disjoint PE row-groups and execute concurrently.
  exp on ACT (PSUM->SBUF bf16), causal diagonal masked in-place on GpSimd.
  AV: attnout^T = v-slice+ones-col (stationary, M=65) P^T (moving); row 64
      accumulates the softmax denominator.
  normalize: reciprocal_approx_fast + K=1 ones-matmul partition-broadcast +
      DVE multiply -> aot bf16; o_proj = Wo-slices (stationary) aot (moving).
"""

import sys

sys.path.insert(0, "/opt/trn_rl_repo")

import numpy as np
import ml_dtypes

import concourse.bass as bass
import concourse.bacc as bacc
import concourse.tile as tile
import concourse.mybir as mybir
from concourse.bass_utils import run_bass_kernel_spmd

F32 = mybir.dt.float32
F32R = mybir.dt.float32r
BF16 = mybir.dt.bfloat16
BF16_NP = ml_dtypes.bfloat16

B, S, D, H = 4, 2048, 1024, 16
HD = D // H  # 64
N_CORES = 8
HPC = 8  # heads per core
J = HPC * HD  # per-core feature dims = 512
NJB = J // 128  # j-blocks per core = 4
NDB = D // 128  # d-blocks = 8
NSB = S // 128  # s-blocks = 16
NC = S // 512  # 512-chunks = 4
VW = 65  # v-tile stride per head: 64 cols + 1 ones col


def build_kernel(debug_dumps=False):
    nc = bacc.Bacc(
        "TRN2", target_bir_lowering=False, debug=False, enable_asserts=False,
        num_devices=N_CORES,
    )

    xt_d = nc.dram_tensor("xt", [D, S], BF16, kind="ExternalInput").ap()
    wq_d = nc.dram_tensor("wq", [D, J], BF16, kind="ExternalInput").ap()
    wk_d = nc.dram_tensor("wk", [D, J], BF16, kind="ExternalInput").ap()
    wv_d = nc.dram_tensor("wv", [D, J], BF16, kind="ExternalInput").ap()
    wo_d = nc.dram_tensor("wo", [J, D], BF16, kind="ExternalInput").ap()
    bq_d = nc.dram_tensor("bq", [128, NJB], F32, kind="ExternalInput").ap()
    bk_d = nc.dram_tensor("bk", [128, NJB], F32, kind="ExternalInput").ap()
    out_d = nc.dram_tensor("out_t", [D, S], F32, kind="ExternalOutput").ap()
    dumps = None
    if debug_dumps:
        dumps = {
            name: nc.dram_tensor(name, shape, BF16, kind="ExternalOutput").ap()
            for name, shape in (
                ("d_qt", [128, NJB * S]),
                ("d_kt", [128, NJB * S]),
                ("d_vt", [128, NSB * HPC * VW]),
                ("d_aot", [128, NJB * S]),
            )
        }

    with tile.TileContext(nc) as tc:
        _emit(tc, nc, xt_d, wq_d, wk_d, wv_d, wo_d, bq_d, bk_d, out_d, dumps)

    nc.compile()
    return nc


def _emit(tc, nc, xt_d, wq_d, wk_d, wv_d, wo_d, bq_d, bk_d, out_d, dumps=None):
    from contextlib import ExitStack

    EXP = mybir.ActivationFunctionType.Exp
    IDENT = mybir.ActivationFunctionType.Identity

    ctx = ExitStack()
    with ctx:
        const = ctx.enter_context(tc.tile_pool(name="const", bufs=1))
        wpool = ctx.enter_context(tc.tile_pool(name="w", bufs=1))
        xpool = ctx.enter_context(tc.tile_pool(name="x", bufs=1))
        qkpool = ctx.enter_context(tc.tile_pool(name="qk", bufs=1))
        vpool = ctx.enter_context(tc.tile_pool(name="v", bufs=1))
        aopool = ctx.enter_context(tc.tile_pool(name="ao", bufs=1))
        ptpool = ctx.enter_context(tc.tile_pool(name="pt", bufs=4))
        nrmpool = ctx.enter_context(tc.tile_pool(name="nrm", bufs=3))
        ogpool = ctx.enter_context(tc.tile_pool(name="og", bufs=2))
        # PSUM: ps slots 3x[128,1024] = 6 banks; acc 2x[65,512] = 2 banks.
        ps = ctx.enter_context(tc.tile_pool(name="ps", bufs=3, space="PSUM"))
        ps_acc = ctx.enter_context(tc.tile_pool(name="ps_acc", bufs=2, space="PSUM"))

        # --- constants ---------------------------------------------------
        # ones columns (bf16) for the v-tile softmax-denominator cols
        ones_v = const.tile([128, NSB * HPC], BF16, tag="ones_v")
        nc.gpsimd.memset(ones_v[:], 1.0)
        # bf16 ones for the recip partition-broadcast matmul; row 64 is used
        # so its base partition matches the denominator row of the acc tiles
        ones_bc = const.tile([65, 64], BF16, tag="ones_bc")
        nc.gpsimd.memset(ones_bc[:], 1.0)
        # causal mask for diagonal 128x128 blocks of S^T[k, q]: keep q >= k
        mask_f = const.tile([128, 128], F32, tag="mask_f")
        nc.gpsimd.memset(mask_f[:], 1.0)
        nc.gpsimd.affine_select(
            mask_f[:], mask_f[:], pattern=[[1, 128]],
            compare_op=mybir.AluOpType.is_ge, fill=0.0,
            base=0, channel_multiplier=-1,
        )
        mask = const.tile([128, 128], BF16, tag="mask")
        nc.vector.tensor_copy(mask[:], mask_f[:])

        # --- weights / biases -------------------------------------------
        w_t = {}
        for name, wd in (("q", wq_d), ("k", wk_d), ("v", wv_d)):
            t = wpool.tile([128, NDB * J], BF16, tag=f"w{name}")
            nc.sync.dma_start(
                t[:].rearrange("p (a j) -> p a j", j=J),
                wd.rearrange("(a p) j -> p a j", p=128),
            )
            w_t[name] = t
        wo_t = wpool.tile([128, NJB * D], BF16, tag="wo")
        nc.sync.dma_start(
            wo_t[:].rearrange("p (a o) -> p a o", o=D),
            wo_d.rearrange("(a p) o -> p a o", p=128),
        )
        bqt = const.tile([128, NJB], F32, tag="bq")
        nc.sync.dma_start(bqt[:], bq_d)
        bkt = const.tile([128, NJB], F32, tag="bk")
        nc.sync.dma_start(bkt[:], bk_d)

        # --- x^T ---------------------------------------------------------
        xt = xpool.tile([128, NDB * S], BF16, tag="xt")
        nc.sync.dma_start(
            xt[:].rearrange("p (a s) -> p a s", s=S),
            xt_d.rearrange("(a p) s -> p a s", p=128),
        )

        # --- v natural [s, j] with ones cols (16 s-block groups) ---------
        # v_t cols: s-block sb at [sb*8*VW, ...), head h at 65h..65h+64,
        # ones at 65h+64.
        v_t = vpool.tile([128, NSB * HPC * VW], BF16, tag="vt")
        nc.vector.tensor_copy(
            v_t[:].rearrange("p (sb h c) -> p (sb h) c", sb=NSB, c=VW)[:, :, 64:65],
            ones_v[:].rearrange("p (a o) -> p a o", o=1),
        )
        for sb in range(NSB):
            pv = ps.tile([128, 1024], F32, tag="ps", name="pv")
            for db in range(NDB):
                nc.tensor.matmul(
                    pv[:, 0:J],
                    xt[:, 2048 * db + 128 * sb: 2048 * db + 128 * (sb + 1)],
                    w_t["v"][:, J * db: J * (db + 1)],
                    start=(db == 0), stop=(db == NDB - 1),
                )
            nc.scalar.activation(
                v_t[:, sb * HPC * VW: (sb + 1) * HPC * VW].rearrange(
                    "p (h c) -> p h c", c=VW)[:, :, 0:64],
                pv[:, 0:J].rearrange("p (h c) -> p h c", c=64),
                IDENT,
            )

        # --- q^T / k^T [j, s] --------------------------------------------
        q_t = qkpool.tile([128, NJB * S], BF16, tag="qt")
        k_t = qkpool.tile([128, NJB * S], BF16, tag="kt")
        for jb in range(NJB):
            for name, dst, bias in (("q", q_t, bqt), ("k", k_t, bkt)):
                for c in range(NC):
                    pq = ps.tile([128, 1024], F32, tag="ps", name="pq")
                    for db in range(NDB):
                        nc.tensor.matmul(
                            pq[:, 0:512],
                            w_t[name][:, J * db + 128 * jb: J * db + 128 * (jb + 1)],
                            xt[:, 2048 * db + 512 * c: 2048 * db + 512 * (c + 1)],
                            start=(db == 0), stop=(db == NDB - 1),
                        )
                    nc.scalar.activation(
                        dst[:, S * jb + 512 * c: S * jb + 512 * (c + 1)],
                        pq[:, 0:512],
                        IDENT,
                        bias=bias[:, jb: jb + 1],
                    )

        # --- attention: 4 head-pairs (j-tiles), 4 q-chunks each ----------
        # Per (pair, chunk, kb): one combined score tile holds head A in cols
        # 0:512 and head B in 512:1024 (separate PSUM banks -> the two K=64
        # matmuls run on disjoint PE row-groups concurrently); one 2D-AP exp
        # covers both heads; AV accumulates per head into [65, 512] accs.
        aot = aopool.tile([128, NJB * S], BF16, tag="aot")
        rows = {"A": slice(0, 64), "B": slice(64, 128)}
        for t in range(NJB):
            head = {"A": 2 * t, "B": 2 * t + 1}
            for c in range(NC):
                acc = {
                    X: ps_acc.tile([65, 512], F32, tag="acc", name=f"acc{X}")
                    for X in ("A", "B")
                }
                n_kb = 4 * c + 4

                def emit_av(pt, kb, wlo):
                    for X in ("A", "B"):
                        nc.tensor.matmul(
                            acc[X][0:65, wlo:512],
                            v_t[:, (8 * kb + head[X]) * VW:
                                (8 * kb + head[X]) * VW + VW],
                            pt[:, 512 * (X == "B") + wlo:
                               512 * (X == "B") + 512],
                            start=(kb == 0), stop=(kb == n_kb - 1),
                        )

                # software-pipelined: AV(kb-1) is emitted AFTER scores(kb) so
                # the in-order PE queue streams scores while ACT runs the exp
                # (an AV waiting on its exp would otherwise block the queue)
                pend = None
                for kb in range(n_kb):
                    wlo = max(0, 128 * kb - 512 * c)
                    st = ps.tile([128, 1024], F32, tag="ps", name="st")
                    for X in ("A", "B"):
                        nc.tensor.matmul(
                            st[:, 512 * (X == "B") + wlo:
                               512 * (X == "B") + 512],
                            k_t[rows[X],
                                S * t + 128 * kb: S * t + 128 * (kb + 1)],
                            q_t[rows[X],
                                S * t + 512 * c + wlo: S * t + 512 * (c + 1)],
                            start=True, stop=True,
                        )
                    pt = ptpool.tile([128, 1024], BF16, tag="pt", name="pt")
                    nc.scalar.activation(
                        pt[:].rearrange("p (x q) -> p x q", x=2)[:, :, wlo:512],
                        st[:].rearrange("p (x q) -> p x q", x=2)[:, :, wlo:512],
                        EXP, scale=0.125,
                    )
                    # causal mask on the diagonal 128x128 block: keep q >= k
                    if kb >= 4 * c:
                        for X in ("A", "B"):
                            o = 512 * (X == "B") + wlo
                            nc.vector.tensor_mul(
                                pt[:, o: o + 128], pt[:, o: o + 128], mask[:],
                            )
                    if pend is not None:
                        emit_av(*pend)
                    pend = (pt, kb, wlo)
                emit_av(*pend)
                # normalize: rows 0..63 raw attnout^T, row 64 = denominator.
                # Broadcast den to partitions 0..63 via a K=1 ones matmul,
                # then reciprocal at base partition 0 (reciprocal_approx_fast
                # mishandles PSUM sources and nonzero base partitions).
                # Head B lands at aot partitions 64..127 via an SBUF->SBUF
                # DMA partition shift (DVE lanes can't cross partitions).
                for X, off in (("A", 0), ("B", 64)):
                    a = acc[X]
                    dnb = nrmpool.tile([65, 512], BF16, tag="dnb")
                    nc.vector.tensor_copy(dnb[64:65, :], a[64:65, :])
                    bcd = ps.tile([64, 512], F32, tag="ps", name="bcd")
                    nc.tensor.matmul(
                        bcd[:], ones_bc[64:65, :], dnb[64:65, :],
                        start=True, stop=True,
                    )
                    dnf = nrmpool.tile([64, 512], F32, tag="dnf")
                    nc.vector.tensor_copy(dnf[:], bcd[:])
                    bct = nrmpool.tile([64, 512], F32, tag="bct")
                    nc.vector.reciprocal_approx_fast(bct[:], dnf[:])
                    if off == 0:
                        nc.vector.tensor_mul(
                            aot[0:64,
                                S * t + 512 * c: S * t + 512 * (c + 1)],
                            a[0:64, :], bct[:],
                        )
                    else:
                        tmp = nrmpool.tile([64, 512], BF16, tag="tmp")
                        nc.vector.tensor_mul(tmp[:], a[0:64, :], bct[:])
                        nc.sync.dma_start(
                            aot[64:128,
                                S * t + 512 * c: S * t + 512 * (c + 1)],
                            tmp[:],
                        )

        if dumps is not None:
            nc.sync.dma_start(dumps["d_qt"][:, :], q_t[:])
            nc.sync.dma_start(dumps["d_kt"][:, :], k_t[:])
            nc.sync.dma_start(dumps["d_vt"][:, :], v_t[:])
            nc.sync.dma_start(dumps["d_aot"][:, :], aot[:])

        # --- o_proj: out^T[o, s] partial = Wo_slice^T @ aot ---------------
        for ob in range(NDB):
            og = ogpool.tile([128, S], F32, tag="og")
            for c in range(NC):
                po = ps.tile([128, 1024], F32, tag="ps", name="po")
                for jb in range(NJB):
                    nc.tensor.matmul(
                        po[:, 0:512],
                        wo_t[:, D * jb + 128 * ob: D * jb + 128 * (ob + 1)],
                        aot[:, S * jb + 512 * c: S * jb + 512 * (c + 1)],
                        start=(jb == 0), stop=(jb == NJB - 1),
                    )
                if c % 2 == 0:
                    nc.scalar.copy(og[:, 512 * c: 512 * (c + 1)], po[:, 0:512])
                else:
                    nc.vector.tensor_copy(og[:, 512 * c: 512 * (c + 1)], po[:, 0:512])
            nc.sync.dma_start(out_d[128 * ob: 128 * (ob + 1), :], og[:])


_NC_CACHE = None


def _get_nc():
    global _NC_CACHE
    if _NC_CACHE is None:
        _NC_CACHE = build_kernel()
    return _NC_CACHE


def build_in_maps(inputs):
    """Host-side sharding: per-core input dict for run_bass_kernel_spmd."""
    x = np.asarray(inputs["hidden_states"], np.float32)
    xt_b = [
        np.ascontiguousarray(x[b].T.astype(BF16_NP)) for b in range(B)
    ]  # [D, S] bf16 per batch
    Wq = np.asarray(inputs["Wq"], np.float32)
    Wk = np.asarray(inputs["Wk"], np.float32)
    Wv = np.asarray(inputs["Wv"], np.float32)
    Wo = np.asarray(inputs["Wo"], np.float32)
    bq = np.asarray(inputs["bq"], np.float32)
    bk = np.asarray(inputs["bk"], np.float32)

    in_maps = []
    for c in range(N_CORES):
        b, g = c // 2, c % 2
        js = slice(g * J, (g + 1) * J)
        in_maps.append({
            "xt": xt_b[b],
            "wq": np.ascontiguousarray(Wq[:, js].astype(BF16_NP)),
            "wk": np.ascontiguousarray(Wk[:, js].astype(BF16_NP)),
            "wv": np.ascontiguousarray(Wv[:, js].astype(BF16_NP)),
            "wo": np.ascontiguousarray(Wo[js, :].astype(BF16_NP)),
            "bq": np.ascontiguousarray(bq[js].reshape(NJB, 128).T),
            "bk": np.ascontiguousarray(bk[js].reshape(NJB, 128).T),
        })
    return in_maps


def assemble_output(results, inputs):
    """Sum per-batch partial pairs, transpose, add folded bias."""
    Wo = np.asarray(inputs["Wo"], np.float32)
    bv = np.asarray(inputs["bv"], np.float32)
    bo = np.asarray(inputs["bo"], np.float32)
    bo_f = bv @ Wo + bo
    out = np.empty((B, S, D), np.float32)
    for b in range(B):
        acc = results[2 * b]["out_t"].astype(np.float32) + \
            results[2 * b + 1]["out_t"].astype(np.float32)
        out[b] = acc.T + bo_f[None, :]
    return out


def kernel(**inputs) -> np.ndarray:
    nc = _get_nc()
    in_maps = build_in_maps(inputs)
    res = run_bass_kernel_spmd(nc, in_maps, core_ids=list(range(N_CORES)))
    return assemble_output(res.results, inputs)


if __name__ == "__main__":
    rng = np.random.default_rng(0)
    ins = {
        "hidden_states": rng.standard_normal((B, S, D)).astype(np.float32),
        "Wq": (rng.standard_normal((D, D)) * 0.02).astype(np.float32),
        "bq": np.zeros(D, np.float32),
        "Wk": (rng.standard_normal((D, D)) * 0.02).astype(np.float32),
        "bk": np.zeros(D, np.float32),
        "Wv": (rng.standard_normal((D, D)) * 0.02).astype(np.float32),
        "bv": np.zeros(D, np.float32),
        "Wo": (rng.standard_normal((D, D)) * 0.02).astype(np.float32),
        "bo": np.zeros(D, np.float32),
    }
    out = kernel(**ins)
    print("out", out.shape, out.dtype, float(np.abs(out).mean()))
